# revision 1
# baseline (speedup 1.0000x reference)
"""Trainium2 Bass kernel for DecouplePreAggGraphConv (GNN message passing).

out[b,j,:] = diag(adj)[j] * (x[b,j] @ W0[j])
           + sum_k offdiag(adj)[j,k] * (x[b,k] @ W1[k])
           + bias

Data-parallel over B across 8 NeuronCores. Per core, per 128-row batch
tile:
  1. one DMA load of x-tile [128, J*128]
  2. PE transposes per joint -> xT_k [n, b] (via identity matmul)
  3. per-joint GEMM  h_k = xT_k.T @ [diag_k*W0_k | W1_k]  -> PSUM [128,256]
  4. drain h to SBUF, then SBUF->SBUF DMA reshuffle into a
     (3-batch-row-group, 35-row) layout: rows = [17 h1 | 17 h0s | bias]
  5. mixing GEMM with a constant block-diagonal [105,51] stationary
     matrix (off.T / I / ones blocks) computes the adjacency mix, the
     self term and the bias add in one pass -> PSUM [51, (g,m)]
  6. drain + one strided store straight into out[b,j,m] layout.
"""

import os
import sys

for _p in ("/opt/trn_rl_repo", "/root/.axon_site/_ro/trn_rl_repo"):
    if os.path.isdir(_p) and _p not in sys.path:
        sys.path.insert(0, _p)

import numpy as np

import concourse.bass as bass
import concourse.mybir as mybir
import concourse.tile as tile
from concourse import bacc
from concourse import bass_utils as _bu
from concourse.bass_utils import run_bass_kernel_spmd

# The folded kernel issues 5 consecutive matmuls per stationary xT_k;
# walrus's default --enable-ldw-opt=false reloads the weights for each.
# Enable the LDW dedupe (output verified against the reference below).
_orig_run_command = _bu.run_command


def _patched_run_command(cmd, *a, **k):
    if isinstance(cmd, list):
        cmd = ["--enable-ldw-opt=true" if c == "--enable-ldw-opt=false"
               else c for c in cmd]
    return _orig_run_command(cmd, *a, **k)


_bu.run_command = _patched_run_command

B, J, FIN, FOUT = 16384, 17, 128, 128
N_CORES = 8
TB = 128            # batch rows per tile
CJ = J * FOUT       # 2176
CJ2 = 2304          # CJ padded to 4.5 PSUM banks so every matmul chunk
                    # is >=256 wide (fp32r full rate) and bank-aligned
G3 = TB // 3        # 42 full groups of 3 rows; rows 126/127 ride as group 42
MAIN = 3 * G3       # 126
NG = G3 + 1         # 43 group slots (last one only has i=0,1 valid)
HPF = NG * FOUT     # 5504 free size of the reshuffled tile
MIXCH = 1024        # mix psum chunk (free elems)
F32 = mybir.dt.float32
BF16 = mybir.dt.bfloat16

_prog_cache: dict[int, object] = {}


def _build_program(bs: int, repeat: int = 1, phases: int = 3):
    """Build the SPMD Bass program for a per-core batch shard of `bs` rows.

    phases (debug/timing only): 1 = stage-1 only, 2 = +bounce, 3 = full.
    """
    nt = bs // TB
    assert bs % TB == 0

    nc = bacc.Bacc("TRN2", target_bir_lowering=False, debug=False,
                   num_devices=N_CORES)

    xs = nc.declare_dram_parameter("xs", [bs, J, FIN], F32, isOutput=False)
    if phases == 0:
        mbig = nc.declare_dram_parameter("mbig", [FIN, J, CJ2],
                                         mybir.dt.float32r, isOutput=False)
        biasbc = nc.declare_dram_parameter("biasbc", [TB, CJ], F32,
                                           isOutput=False)
    else:
        wcat = nc.declare_dram_parameter("wcat", [FIN, J, 2 * FOUT], F32,
                                         isOutput=False)
        mix3 = nc.declare_dram_parameter("mix3", [105, 51], BF16,
                                         isOutput=False)
        bias43 = nc.declare_dram_parameter("bias43", [3, HPF], BF16,
                                           isOutput=False)
    ident = nc.declare_dram_parameter("ident", [128, 128], F32, isOutput=False)
    out = nc.declare_dram_parameter("out", [bs, J, FOUT], F32, isOutput=True)

    if phases == 0:
        return _build_folded(nc, xs, mbig, biasbc, ident, out, bs, repeat)

    with tile.TileContext(nc) as tc:
        with (
            tc.tile_pool(name="const", bufs=1) as cpool,
            tc.tile_pool(name="x", bufs=2) as xpool,
            tc.tile_pool(name="xt", bufs=3) as xtpool,
            tc.tile_pool(name="hsb", bufs=2) as hpool,
            tc.tile_pool(name="hp", bufs=2) as hppool,
            tc.tile_pool(name="osb", bufs=2) as opool,
            tc.tile_pool(name="tp", bufs=2, space=bass.MemorySpace.PSUM) as tpp,
            tc.tile_pool(name="hps", bufs=2, space=bass.MemorySpace.PSUM) as hpsp,
            tc.tile_pool(name="mxp", bufs=2, space=bass.MemorySpace.PSUM) as mxpp,
        ):
            # ---- constants, loaded once ----
            wcat_sb = cpool.tile([FIN, J, 2 * FOUT], F32, tag="wcat")
            nc.sync.dma_start(wcat_sb[:], wcat[:])
            mix3_sb = cpool.tile([105, 51], BF16, tag="mix3")
            nc.sync.dma_start(mix3_sb[:], mix3[:])
            id_sb = cpool.tile([128, 128], F32, tag="ident")
            nc.sync.dma_start(id_sb[:], ident[:])

            # ping-pong DRAM scratch for the reshuffle bounce; the
            # (i=2, g=42) rectangle never gets scattered into, so zero it
            # once (PE accumulates 0*garbage = NaN otherwise).
            scrs = [nc.dram_tensor(f"scr{p}", [102, HPF], BF16)
                    for p in range(2)]
            zro = cpool.tile([34, FOUT], BF16, tag="zro")
            nc.gpsimd.memset(zro[:], 0.0)
            for p in range(2):
                nc.sync.dma_start(
                    scrs[p][68:102, G3 * FOUT:], zro[:])

            for t in range(nt * repeat):
                t = t % nt
                b0 = t * TB
                # 1. load x tile
                x_t = xpool.tile([TB, J, FIN], F32, tag="x")
                nc.sync.dma_start(x_t[:], xs[b0:b0 + TB])

                # 2/3/4a. per joint: transpose, GEMM, drain (cast bf16)
                h_sb = hpool.tile([TB, 2, J, FOUT], BF16, tag="h")
                for k in range(J):
                    tp = tpp.tile([128, TB], F32, tag="tp")
                    nc.tensor.transpose(tp[:], x_t[:, k, :], id_sb[:])
                    xt = xtpool.tile([128, TB], F32, tag="xt")
                    if k % 2 == 0:
                        nc.vector.tensor_copy(xt[:], tp[:])
                    else:
                        nc.scalar.copy(xt[:], tp[:])
                    hk = hpsp.tile([TB, 2 * FOUT], F32, tag="hk")
                    nc.tensor.matmul(hk[:], xt[:], wcat_sb[:, k, :])
                    if k % 2 == 0:
                        nc.scalar.copy(h_sb[:, :, k, :], hk[:])
                    else:
                        nc.vector.tensor_copy(h_sb[:, :, k, :], hk[:])

                # 4b. reshuffle via DRAM bounce: scatter h into the group
                # layout in a DRAM scratch (rows r = i*34 + h*17 + k), then
                # read it back contiguously. DRAM APs have no partition-dim
                # restriction, so this is 3 scatter DMAs + 2 readback DMAs.
                if phases == 1:
                    nc.sync.dma_start(
                        out[b0:b0 + TB].rearrange("b j m -> b (j m)")
                        .bitcast(BF16)[:, :J * FOUT],
                        h_sb[:, 0])
                    continue
                scr = scrs[t % 2]
                sv = scr.rearrange("(i h k) (g m) -> i g h k m",
                                   i=3, h=2, k=17, g=NG, m=FOUT)
                for i in range(3):
                    ng = NG if i < 2 else G3
                    nc.sync.dma_start(sv[i, :ng], h_sb[i:TB:3])
                hp_t = hppool.tile([105, HPF], BF16, tag="hp")
                nc.sync.dma_start(hp_t[0:102, :], scr[:])
                nc.sync.dma_start(hp_t[102:105, :], bias43[:])
                if phases == 2:
                    nc.sync.dma_start(
                        out[b0:b0 + 105].rearrange("b j m -> b (j m)")
                        .bitcast(BF16)[:, :43],
                        hp_t[:, 0:43])
                    continue

                # 5/6. mix GEMM chunks, drain, store
                # out[(i,j),(g,m)] = h0s[3g+i,j,m]
                #                  + sum_k off[j,k]*h1[3g+i,k,m] + bias[m]
                # (i=2, g=42) columns are garbage and never stored.
                o_sb = opool.tile([51, HPF], F32, tag="osb")
                nch = (HPF + MIXCH - 1) // MIXCH
                for c in range(nch):
                    f0 = c * MIXCH
                    fw = min(MIXCH, HPF - f0)
                    mp = mxpp.tile([51, MIXCH], F32, tag="mx")
                    for s0 in range(0, fw, 512):
                        sw = min(512, fw - s0)
                        nc.tensor.matmul(mp[:, s0:s0 + sw], mix3_sb[:],
                                         hp_t[:, f0 + s0:f0 + s0 + sw])
                    if c % 2 == 0:
                        nc.vector.tensor_copy(o_sb[:, f0:f0 + fw], mp[:, :fw])
                    else:
                        nc.scalar.copy(o_sb[:, f0:f0 + fw], mp[:, :fw])

                dst = out[b0:b0 + MAIN].rearrange("(g i) j m -> i j g m", i=3)
                nc.sync.dma_start(dst, o_sb[:, :G3 * FOUT])
                nc.sync.dma_start(out[b0 + MAIN:b0 + TB],
                                  o_sb[0:34, G3 * FOUT:])

    nc.compile()
    return nc


def _build_folded(nc, xs, mbig, biasbc, ident, out, bs, repeat):
    """Single folded GEMM: out[b,(j,m)] = x[b,(k,n)] @ Mbig + bias.

    Mbig[(k,n),(j,m)] = off[j,k]*W1[k,n,m] + (k==j)*diag[j]*W0[j,n,m].
    2 DMAs per tile; PE streams 17 x 2176 columns per 128-row tile.
    """
    nt = bs // TB
    with tile.TileContext(nc) as tc:
        with (
            tc.tile_pool(name="const", bufs=1) as cpool,
            tc.tile_pool(name="x", bufs=2) as xpool,
            tc.tile_pool(name="xt", bufs=18) as xtpool,
            tc.tile_pool(name="osb", bufs=1) as opool,
            tc.tile_pool(name="tp", bufs=2, space=bass.MemorySpace.PSUM) as tpp,
            tc.tile_pool(name="of", bufs=1, space=bass.MemorySpace.PSUM) as ofp,
        ):
            F32R = mybir.dt.float32r
            mb_sb = cpool.tile([FIN, J, CJ2], F32R, tag="mbig")
            nc.sync.dma_start(mb_sb[:], mbig[:])
            bb_sb = cpool.tile([TB, CJ], F32, tag="biasbc")
            nc.sync.dma_start(bb_sb[:], biasbc[:])
            id_sb = cpool.tile([128, 128], F32, tag="ident")
            nc.sync.dma_start(id_sb[:], ident[:])

            # fp32 matmul streams at 4 cycles/row on TRN2; float32r (same
            # bits, reduced-precision multiply) streams at 1 cycle/row for
            # N >= 256. Chunks must also stay inside single 2KB PSUM banks
            # (512 f32): four aligned 512-wide chunks + a 256-wide tail
            # into the zero-padded 2176:2304 region.
            chunks = [(0, 512), (512, 512), (1024, 512), (1536, 512),
                      (2048, 256)]
            for t in range(nt * repeat):
                t = t % nt
                b0 = t * TB
                x_t = xpool.tile([TB, J, FIN], F32, tag="x")
                nc.sync.dma_start(x_t[:], xs[b0:b0 + TB])

                # transposes first: PE fills the wait for the previous
                # tile's PSUM drain with them before touching `of`.
                xts = []
                for k in range(J):
                    tp = tpp.tile([128, TB], F32, tag="tp")
                    nc.tensor.transpose(tp[:], x_t[:, k, :], id_sb[:])
                    xt = xtpool.tile([128, TB], F32R, tag="xt")
                    if k % 2 == 0:
                        nc.vector.tensor_copy(xt[:], tp[:])
                    else:
                        nc.scalar.copy(xt[:], tp[:])
                    xts.append(xt)
                of = ofp.tile([TB, CJ2], F32, tag="of")
                for k in range(J):
                    for c0, cw in chunks:
                        nc.tensor.matmul(of[:, c0:c0 + cw], xts[k][:],
                                         mb_sb[:, k, c0:c0 + cw],
                                         start=(k == 0), stop=(k == J - 1))

                o_sb = opool.tile([TB, CJ], F32, tag="osb")
                half = CJ // 2
                nc.vector.tensor_add(o_sb[:, :half], of[:, :half],
                                     bb_sb[:, :half])
                nc.vector.tensor_add(o_sb[:, half:], of[:, half:CJ],
                                     bb_sb[:, half:])
                nc.sync.dma_start(
                    out[b0:b0 + TB].rearrange("b j m -> b (j m)"), o_sb[:])

    nc.compile()
    return nc


def _host_prep(x, W, bias, adj, bs):
    """Build the per-core input maps."""
    diag = np.diagonal(adj).astype(np.float32)
    off = (adj * (1.0 - np.eye(J, dtype=adj.dtype))).astype(np.float32)

    # stage-1 weights: [FIN, J, 2*FOUT], columns = [diag_k*W0_k | W1_k]
    wcat = np.concatenate([diag[:, None, None] * W[0], W[1]], axis=2)
    wcat = np.ascontiguousarray(wcat.transpose(1, 0, 2)).astype(np.float32)

    # mixing stationary: rows r = i*34 + h*17 + k (h=0: h0s, h=1: h1),
    # rows 102+i: bias; cols (i'*17 + j)
    import ml_dtypes
    mixblock = np.zeros((34, J), dtype=np.float32)
    mixblock[0:J, :] = np.eye(J, dtype=np.float32)  # h0s rows
    mixblock[J:2 * J, :] = off.T      # h1 rows: sum_k off[j,k] h1_k
    mix3 = np.zeros((105, 51), dtype=np.float32)
    for i in range(3):
        mix3[i * 34:(i + 1) * 34, i * J:(i + 1) * J] = mixblock
        mix3[102 + i, i * J:(i + 1) * J] = 1.0      # bias row

    bias43 = np.tile(bias.astype(np.float32), (3, NG))
    ident = np.eye(128, dtype=np.float32)

    # folded weights: Mbig[(k,n),(j,m)], stored n-partition-major
    m4 = off.T[:, :, None, None] * W[1][:, None, :, :]   # [k, j, n, m]
    m4[np.arange(J), np.arange(J)] += diag[:, None, None] * W[0]
    mbig = m4.transpose(0, 2, 1, 3).reshape(J * FIN, CJ)  # rows (k,n)
    mbig = np.ascontiguousarray(
        mbig.reshape(J, FIN, CJ).transpose(1, 0, 2)).astype(np.float32)
    mbig = np.concatenate(
        [mbig, np.zeros((FIN, J, CJ2 - CJ), np.float32)], axis=2)

    shared = {
        "wcat": wcat,
        "mix3": mix3.astype(ml_dtypes.bfloat16),
        "bias43": np.ascontiguousarray(bias43).astype(ml_dtypes.bfloat16),
        "ident": ident,
        "mbig": mbig,
        "biasbc": np.ascontiguousarray(np.broadcast_to(
            np.tile(bias.astype(np.float32), 17), (TB, CJ))),
    }
    in_maps = []
    for c in range(N_CORES):
        m = dict(shared)
        m["xs"] = np.ascontiguousarray(x[c * bs:(c + 1) * bs])
        in_maps.append(m)
    return in_maps


def _run(x, W, bias, adj, bs, profile=False, tmpdir=None, phases=0):
    key = (bs, phases)
    if key not in _prog_cache:
        _prog_cache[key] = _build_program(bs, phases=phases)
    nc = _prog_cache[key]
    in_maps = _host_prep(x, W, bias, adj, bs)
    res = run_bass_kernel_spmd(nc, in_maps, list(range(N_CORES)),
                               trace=profile, tmpdir=tmpdir)
    out = np.concatenate([res.results[c]["out"] for c in range(N_CORES)],
                         axis=0)
    if profile:
        return out, res
    return out


def kernel(x, W, bias, adj):
    x = np.asarray(x, dtype=np.float32)
    W = np.asarray(W, dtype=np.float32)
    bias = np.asarray(bias, dtype=np.float32)
    adj = np.asarray(adj, dtype=np.float32)
    assert x.shape == (B, J, FIN)
    return _run(x, W, bias, adj, B // N_CORES)



# revision 2
# speedup vs baseline: 1.0623x; 1.0623x over previous
"""Trainium2 Bass kernel for DecouplePreAggGraphConv (GNN message passing).

out[b,j,:] = diag(adj)[j] * (x[b,j] @ W0[j])
           + sum_k offdiag(adj)[j,k] * (x[b,k] @ W1[k])
           + bias

Data-parallel over B across 8 NeuronCores. Low-FLOP ("smart") algorithm:
per-joint GEMMs (K=128) for h0/h1, then the 17x17 adjacency mix fused
into one small stationary matmul per m-triple group -- no DRAM bounce.

Per core, per 128-row batch tile:
  1. x arrives host-pretransposed/bf16 as xT[j, n, b]; one DMA loads
     [n, j, b]-layout tiles directly (no PE transposes of x).
  2. stage-1 per joint k: one matmul -> PSUM [b, 258], columns
     pre-ordered (t, h, dm) so the whole tile drains in ONE copy into
     h_sb[b, t, h, c3] bf16 (c = h*64 + k*3 + dm, m = 3t+dm)
  3. h_sb pad columns 54:57 hold bias[3t+dm] (written once per buffer),
     so the per-triple PE transposes ([b,128] -> [128, b]) carry bias
     rows into hT for free
  4. one matmul per 4-triple group with stationary bigmix [118, 51]:
     self term + adjacency mix + bias for 51 (j,dm') outputs at once
  5. drain 4 groups per 2-bank PSUM tile into o_sb [128, 2944]; one
     contiguous bf16 store per tile; host un-permutes + upcasts.
"""

import os
import sys

for _p in ("/opt/trn_rl_repo", "/root/.axon_site/_ro/trn_rl_repo"):
    if os.path.isdir(_p) and _p not in sys.path:
        sys.path.insert(0, _p)

import numpy as np

import concourse.bass as bass
import concourse.mybir as mybir
import concourse.tile as tile
from concourse import bacc
from concourse import bass_utils as _bu
from concourse.bass_utils import run_bass_kernel_spmd

B, J, FIN, FOUT = 16384, 17, 128, 128
N_CORES = 8
TB = 128              # batch rows per tile
NT3 = 43              # m-triples per tile (128 = 3*43 - 1; (42,2) is pad)
NG = 11               # groups of <=4 triples: 10 full + 1 of 3 triples
GW = [512] * 10 + [384]          # mix free width per group
GOFF = [512 * g for g in range(11)]
CDIM = 128            # transpose block: c = h*64 + k*3 + dm (+pads)
MIXK = 118            # mix contraction rows (bias rides c=54:57)
OFREE = 6 * 512 - 128  # o_sb free size: 5 paired blocks + 384 tail = 2944
ROWS2 = 128            # o_sb partitions: group pair at rows 0:51 and 64:115
F32 = mybir.dt.float32
BF16 = mybir.dt.bfloat16

_prog_cache: dict[tuple, object] = {}


def _build_program(bs: int, repeat: int = 1):
    """Build the SPMD Bass program for a per-core batch shard of `bs` rows."""
    nt = bs // TB
    assert bs % (2 * TB) == 0, "bs must be a multiple of 256 (paired tiles)"
    np2 = nt // 2

    nc = bacc.Bacc("TRN2", target_bir_lowering=False, debug=False,
                   num_devices=N_CORES)

    xt = nc.declare_dram_parameter("xt", [J, FIN, bs], BF16, isOutput=False)
    wcat = nc.declare_dram_parameter("wcat", [FIN, J, 258], BF16,
                                     isOutput=False)
    bigmix = nc.declare_dram_parameter("bigmix", [MIXK, 51], BF16,
                                       isOutput=False)
    biash = nc.declare_dram_parameter("biash", [TB, NT3, 3], BF16,
                                      isOutput=False)
    ident = nc.declare_dram_parameter("ident", [128, 128], BF16,
                                      isOutput=False)
    outT = nc.declare_dram_parameter("outT", [nt, ROWS2, OFREE], BF16,
                                     isOutput=True)

    with tile.TileContext(nc) as tc:
        with (
            tc.tile_pool(name="const", bufs=1) as cpool,
            tc.tile_pool(name="x", bufs=2) as xpool,
            tc.tile_pool(name="h", bufs=2) as hpool,
            tc.tile_pool(name="hT", bufs=3) as hTpool,
            tc.tile_pool(name="o", bufs=2) as opool,
            tc.tile_pool(name="hk", bufs=2, space=bass.MemorySpace.PSUM) as hkp,
            tc.tile_pool(name="tp", bufs=2, space=bass.MemorySpace.PSUM) as tpp,
            tc.tile_pool(name="mx", bufs=1, space=bass.MemorySpace.PSUM) as mxp,
        ):
            # ---- constants, loaded once ----
            wcat_sb = cpool.tile([FIN, J, 258], BF16, tag="wcat")
            nc.sync.dma_start(wcat_sb[:], wcat[:])
            bigmix_sb = cpool.tile([MIXK, 51], BF16, tag="bigmix")
            nc.sync.dma_start(bigmix_sb[:], bigmix[:])
            id_sb = cpool.tile([128, 128], BF16, tag="ident")
            nc.sync.dma_start(id_sb[:], ident[:])

            # engine-aware PSUM drains (gpsimd can't see PSUM):
            # bf16->bf16 hT drains run 2x on DVE; f32-source drains are
            # 1x everywhere, so spread them to balance busy time.
            def drain(dst, src, kind):
                if kind in ("hT", "hD"):
                    eng = nc.vector.tensor_copy
                else:  # "o" / "hA"
                    eng = nc.scalar.copy
                eng(dst, src)

            def stage1(x_t, half):
                # h_sb[b, t, h, c3], c3 = k*3 + dm; c3 = 51:54 (k=17
                # slot) and 57:64 / h=1 tail stay zero, c3 = 54:57 of
                # h=0 holds bias (both written once below).
                # Joints are processed in pairs: both matmuls of a pair
                # land in one 2-bank PSUM tile and drain in ONE copy.
                h_sb = hpool.tile([TB, NT3, 2, 64], BF16, tag="h")
                for p in range(9):
                    ks = [k for k in (2 * p, 2 * p + 1) if k < J]
                    hk = hkp.tile([TB, 2, 512], F32, tag="hk")
                    for k2, k in enumerate(ks):
                        nc.tensor.matmul(
                            hk[:, k2, 0:258],
                            x_t[:, k, half * TB:(half + 1) * TB],
                            wcat_sb[:, k, :])
                    if len(ks) == 2:
                        dst = h_sb[:, :, :, 6 * p:6 * p + 6].rearrange(
                            "b t h (k2 dm) -> b k2 t h dm", k2=2)
                        drain(dst, hk[:, :, 0:258],
                              "hA" if p < 6 else "hD")
                    else:
                        drain(h_sb[:, :, :, 6 * p:6 * p + 3],
                              hk[:, 0, 0:258], "hD")
                return h_sb

            def mix(h_sb, t_out):
                o_sb = opool.tile([ROWS2, OFREE], BF16, tag="o")
                for g4 in range(3):
                    mp = mxp.tile([ROWS2, 1024], F32, tag="mx")
                    bw = 0
                    for g in range(4 * g4, min(4 * g4 + 4, NG)):
                        w = GW[g]
                        sub, c0 = g % 2, ((g // 2) % 2) * 512
                        tp = tpp.tile([CDIM, 512], BF16, tag="tp")
                        for ts in range(w // TB):
                            nc.tensor.transpose(
                                tp[:, ts * TB:(ts + 1) * TB],
                                h_sb[:, 4 * g + ts, :, :], id_sb[:])
                        hT = hTpool.tile([CDIM, 512], BF16, tag="hT")
                        drain(hT[:, :w], tp[:, :w], "hT")
                        nc.tensor.matmul(
                            mp[sub * 64:sub * 64 + 51, c0:c0 + w],
                            bigmix_sb[:], hT[0:MIXK, :w])
                        bw = c0 + w
                    drain(o_sb[:, g4 * 1024:g4 * 1024 + bw], mp[:, :bw],
                          "o")
                nc.sync.dma_start(outT[t_out], o_sb[:])

            # one-time h-buffer init: zero pads (0*NaN = NaN in the mix
            # matmul otherwise) and plant the bias columns at h=0 c3=54:57
            for _ in range(2):
                hz = hpool.tile([TB, NT3, 2, 64], BF16, tag="h")
                nc.gpsimd.memset(hz[:], 0.0)
                nc.sync.dma_start(hz[:, :, 0, 54:57], biash[:])

            # software pipeline: stage1(t) runs on PE while the copy
            # engines drain t's h; mix(t-1) fills the PE meanwhile.
            prev = None
            for it in range(np2 * repeat):
                p = it % np2
                x_t = xpool.tile([FIN, J, 2 * TB], BF16, tag="x")
                nc.sync.dma_start(
                    x_t[:],
                    xt[:, :, p * 2 * TB:(p + 1) * 2 * TB]
                    .rearrange("j n b -> n j b"))
                for half in (0, 1):
                    h_new = stage1(x_t, half)
                    if prev is not None:
                        mix(*prev)
                    prev = (h_new, 2 * p + half)
            if prev is not None:
                mix(*prev)

    nc.compile()
    return nc


def _host_prep(x, W, bias, adj, bs):
    """Build the per-core input maps (weights-only compute + layout)."""
    import ml_dtypes
    diag = np.diagonal(adj).astype(np.float32)
    off = (adj * (1.0 - np.eye(J, dtype=adj.dtype))).astype(np.float32)

    # stage-1 weights [FIN, J, 258]: col q = t*6 + h*3 + dm holds
    # (h==0 ? diag_k*W0_k : W1_k)[:, 3t+dm], zero at the m=128 pad
    wh = np.zeros((J, FIN, 2, 129), np.float32)
    wh[:, :, 0, :128] = diag[:, None, None] * W[0]
    wh[:, :, 1, :128] = W[1]
    wc = wh.reshape(J, FIN, 2, 43, 3).transpose(0, 1, 3, 2, 4)
    wcat = np.ascontiguousarray(wc.reshape(J, FIN, 258).transpose(1, 0, 2))

    # mix stationary [118, 51]: rows c = h*64 + k*3 + dm (pads zero),
    # rows 54:57 = bias pass-through; cols p = j*3 + dm'
    bm = np.zeros((MIXK, 51), np.float32)
    bm[np.arange(51), np.arange(51)] = 1.0          # h0 self rows
    for dm in range(3):
        for k in range(J):
            bm[64 + 3 * k + dm, dm::3] = off[:, k]  # h1 mix rows
        bm[54 + dm, dm::3] = 1.0                    # bias rows
    # bias plane [TB, 43, 3]: biash[b, t, dm] = bias[3t+dm] (b-bcast)
    mvals = 3 * np.arange(NT3)[:, None] + np.arange(3)[None, :]
    bvals = np.where(mvals < FOUT, bias[np.minimum(mvals, FOUT - 1)], 0.0)
    biash = np.ascontiguousarray(
        np.broadcast_to(bvals[None], (TB, NT3, 3))).astype(np.float32)

    shared = {
        "wcat": wcat.astype(ml_dtypes.bfloat16),
        "bigmix": bm.astype(ml_dtypes.bfloat16),
        "biash": biash.astype(ml_dtypes.bfloat16),
        "ident": np.eye(128, dtype=np.float32).astype(ml_dtypes.bfloat16),
    }
    in_maps = []
    for c in range(N_CORES):
        m = dict(shared)
        xs = x[c * bs:(c + 1) * bs]                  # [bs, J, FIN]
        m["xt"] = np.ascontiguousarray(
            xs.transpose(1, 2, 0)).astype(ml_dtypes.bfloat16)
        in_maps.append(m)
    return in_maps


_decode_idx_cache: dict[int, np.ndarray] = {}


def _decode_idx():
    """Flat gather indices: out[b,j,m] = outT_flat[tile, idx[j,m] + b]."""
    if 0 not in _decode_idx_cache:
        idx = np.zeros((J, FOUT), np.int64)
        for m in range(FOUT):
            t, dm = divmod(m, 3)
            g, ts = divmod(t, 4)
            for j in range(J):
                if g < 10:
                    row = (g % 2) * 64 + j * 3 + dm
                    col = (g // 2) * 512 + ts * TB
                else:
                    row = j * 3 + dm
                    col = 2560 + ts * TB
                idx[j, m] = row * OFREE + col
        _decode_idx_cache[0] = idx
    return _decode_idx_cache[0]


def _decode_out(outT_core, bs):
    """[nt, 102, OFREE] bf16 -> [bs, J, FOUT] f32."""
    nt = bs // TB
    flat = np.ascontiguousarray(outT_core).reshape(nt, ROWS2 * OFREE)
    idx = _decode_idx()                      # [J, FOUT]
    gather = flat[:, idx[None, :, :, None] +
                  np.arange(TB)[None, None, None, :]]  # [nt,1? J,FOUT,TB]
    gather = gather.reshape(nt, J, FOUT, TB)
    return np.ascontiguousarray(
        gather.transpose(0, 3, 1, 2)).reshape(bs, J, FOUT).astype(np.float32)


def _run(x, W, bias, adj, bs, profile=False, tmpdir=None):
    key = (bs,)
    if key not in _prog_cache:
        _prog_cache[key] = _build_program(bs)
    nc = _prog_cache[key]
    in_maps = _host_prep(x, W, bias, adj, bs)
    res = run_bass_kernel_spmd(nc, in_maps, list(range(N_CORES)),
                               trace=profile, tmpdir=tmpdir)
    out = np.concatenate(
        [_decode_out(res.results[c]["outT"], bs) for c in range(N_CORES)],
        axis=0)
    if profile:
        return out, res
    return out


def kernel(x, W, bias, adj):
    x = np.asarray(x, dtype=np.float32)
    W = np.asarray(W, dtype=np.float32)
    bias = np.asarray(bias, dtype=np.float32)
    adj = np.asarray(adj, dtype=np.float32)
    assert x.shape == (B, J, FIN)
    return _run(x, W, bias, adj, B // N_CORES)
